# revision 3
# baseline (speedup 1.0000x reference)
"""Trainium2 Bass kernel for nn_AttentionModule_39616778338491 (chord sparse attention).

Structure: V = gMLP(V); 12x { W = fMLP_m(input); V = chord_spmm(W, V) + V }.

Sharding (8 cores): core c = 2b+h -> batch b. f-MLPs are data-parallel over
row halves h; ONE concatenated pair-AllGather shares all 12 layers' W. The
g-MLP and the chord chain are E-split: each core computes the FULL 4096-row
chain on its own 128-column E-half (chord is independent across E), so no V
AllGather and no per-layer halo exchange exist at all.

Program layout: TileContext 1 (12 f-MLPs + the g-MLP interleaved, bf16
matmuls; V crosses to phase 2 via a DRAM roundtrip) -> raw gpsimd block
issuing the split AllGather (no wait; waits patched onto the first wsos
readers post-scheduling) -> all-engine barrier -> TileContext 2 (chord
chain only).

f-mm2 ([rows,1024]@[1024,13]) is packed 4 chunks deep into the PE via
column tiling (tile_position=(0,32j)): one 32-matmul accumulation group
computes all four chunks' W concurrently, 4x fewer PE cycles than M=13
alone. The +V residual is folded into the self-link weight on host.

Chord spmm as dense PE matmuls: per 128-row output block the 13 power-of-2
offsets touch 6 source blocks {+0,+1,+2,+4,+8,+16}. Slots +0/+1 (links
d<=128, with wraparound spill) are bf16 lhsT images built via skewed flat
DMAs into DRAM (diagonal writes couple partition and byte offsets, which
only DRAM-side APs allow) and reloaded. The four single-diagonal slots
(d=256..2048) never touch DRAM: a [128,4*32] weight grid is formed by small
SBUF->SBUF DMAs from the interleaved W tile, then DVE/GpSimd tensor_muls
against a broadcast identity write the diagonal lhsT tiles in SBUF.
PSUM is evacuated straight into per-group bf16 tiles (the f32 V buffer of
the old design carried no extra precision - the matmul inputs are bf16
either way), so the next layer's matmuls chase the evacuations group by
group instead of waiting on a full-width copy. Prep runs as a 2-stage
software pipeline (skew writes for layer m+2, S loads/D builds for m+1)
with each layer's DRAM staging FIFO-ordered on one of two HWDGE rings.
"""

import os
import numpy as np

B, N, E, H = 4, 4096, 256, 1024
NW = 12
NL = 13
OFFS = [0, 1, 2, 4, 8, 16, 32, 64, 128, 256, 512, 1024, 2048]
ROWS = N // 2          # rows per core for f-MLP work
NBLK = N // 128        # 32 blocks of 128 rows
EH = E // 2            # e-half per core for g + chord
CH = 512               # row-chunk for MLP matmuls
NCH = ROWS // CH       # f-MLP chunks (own half)
NCHG = N // CH         # g-MLP chunks (full N)
HT = H // 128          # 8 h-tiles
PITCH = NBLK * 128     # free width of an S/D tile (elems)
GROUPS = [[0, 1], [2, 3], [4, 5], [6, 7]]
DIAG_BOFF = [2, 4, 8, 16]   # links 9..12 (d=256..2048): pure diagonals
NG = NBLK // 4         # 8 evac groups of 4 blocks


def _install_patches():
    """Walrus in this image rejects >1 sem wait on the Tile tail Drain;
    spread the waits across preceding sync-engine nops. Also raise the
    stale SBUF cap (207.87 KB/partition is the real limit here)."""
    import concourse.mybir as mybir
    from concourse.tile import TileContext
    from concourse.vector_clock import ScopedClock
    from concourse import tile_utils

    def _dab(self, tick_clock, wait_clock):
        nops = [self.nc.sync.nop(nofuse=True) for _ in range(27)]
        drain_inst = self.nc.sync.drain()
        wait_clock.add_sem_waits(
            drain_inst.ins, ScopedClock({None: tick_clock.global_clock})
        )
        si = drain_inst.ins.sync_info
        waits = list(si.on_wait) if si else []
        if len(waits) > 1:
            si.on_wait.clear()
            si.on_wait.append(waits[0])
            for w, nop in zip(waits[1:], nops):
                nsi = nop.ins.sync_info
                if nsi is None:
                    nop.ins.sync_info = mybir.SyncInfo(on_update=[], on_wait=[w])
                else:
                    nsi.on_wait.append(w)
        self.nc.all_engine_barrier()
        popped = self.nc._tile_sem_poison_stack.pop()
        assert popped is self._sem_poison
        self.nc.clear_and_free_semaphores(list(self.sems.allocated().values()))
        self.nc.all_engine_barrier()

    TileContext._drain_and_barrier = _dab
    tile_utils.max_sbuf_usage = 206 * 1024


def _split_multi_waits(nc, mybir, limit=1):
    """This walrus build accepts at most one sem wait per instruction;
    hoist extra waits onto same-engine NoOps inserted just before."""
    uid = 0
    for f in nc.m.functions:
        for bb in f.blocks:
            new = []
            for inst in bb.instructions:
                si = inst.sync_info
                waits = list(si.on_wait) if si and si.on_wait else []
                if len(waits) > limit:
                    for w in waits[:-limit]:
                        nop = mybir.InstNoOp(name=f"waitsplit-{uid}", ins=[], outs=[])
                        uid += 1
                        nop.engine = inst.engine
                        nop.sync_info = mybir.SyncInfo(on_update=[], on_wait=[w])
                        new.append(nop)
                    si.on_wait.clear()
                    si.on_wait.append(waits[-1])
                new.append(inst)
            bb.instructions = new


def _build_program(nw):
    import bass_rust
    import concourse.bass as bass
    import concourse.mybir as mybir
    from concourse.tile import TileContext

    f32 = mybir.dt.float32
    bf16 = mybir.dt.bfloat16
    AF = mybir.ActivationFunctionType
    V64 = bass_rust.VecI64Pair

    nc = bass.Bass()
    vtf = nc.declare_dram_parameter("vtf", [E, N], bf16, isOutput=False)
    inpt = nc.declare_dram_parameter("inpt", [E, ROWS], bf16, isOutput=False)
    gw1 = nc.declare_dram_parameter("gw1", [E, H], bf16, isOutput=False)
    gw2h = nc.declare_dram_parameter("gw2h", [H, EH], bf16, isOutput=False)
    gb1t = nc.declare_dram_parameter("gb1t", [128, HT], f32, isOutput=False)
    gb2h = nc.declare_dram_parameter("gb2h", [1, EH], bf16, isOutput=False)
    fw1 = nc.declare_dram_parameter("fw1", [nw, E, H], bf16, isOutput=False)
    fw2t = nc.declare_dram_parameter("fw2t", [nw, 128, HT * NL], bf16, isOutput=False)
    fb1t = nc.declare_dram_parameter("fb1t", [128, nw * HT], f32, isOutput=False)
    fb2r = nc.declare_dram_parameter("fb2r", [128, nw], f32, isOutput=False)
    onesr = nc.declare_dram_parameter("onesr", [1, 128], bf16, isOutput=False)
    idmr = nc.declare_dram_parameter("idmr", [128, 128], bf16, isOutput=False)
    out = nc.declare_dram_parameter("out", [N, EH], f32, isOutput=True)

    # raw DRAM staging (crosses the TileContext boundary; the phase barrier
    # orders accesses)
    split = min(2, nw)
    wsis_all = nc.dram_tensor("wsis_all", [nw, NL, ROWS], bf16)
    wsos_a = nc.dram_tensor("wsos_a", [2, split, NL, ROWS], bf16)
    wsos_b = (
        nc.dram_tensor("wsos_b", [2, nw - split, NL, ROWS], bf16)
        if nw > split else None
    )
    stage = [nc.dram_tensor(f"sst{p}", [2 * 128 * PITCH], bf16) for p in range(2)]
    vg = nc.dram_tensor("vg", [128, NBLK * EH], bf16)

    # ---------------- phase 1: f MLPs + g MLP ----------------
    with TileContext(nc) as tc:
        with (
            tc.tile_pool(name="pc", bufs=1) as pc,
            tc.tile_pool(name="pin", bufs=1) as pin,
            tc.tile_pool(name="pfh", bufs=1) as pfh,
            tc.tile_pool(name="pgh", bufs=1) as pgh,
            tc.tile_pool(name="pfw1", bufs=2) as pfw1,
            tc.tile_pool(name="pfw2", bufs=2) as pfw2,
            tc.tile_pool(name="ptmp", bufs=2) as ptmp,
            tc.tile_pool(name="pvg", bufs=1) as pvg,
            tc.tile_pool(name="psA", bufs=3, space="PSUM") as psA,
            tc.tile_pool(name="psW", bufs=2, space="PSUM") as psW,
            tc.tile_pool(name="psG", bufs=2, space="PSUM") as psG,
        ):
            fb1_t = pc.tile([128, nw * HT], f32, tag="fb1", name="fb1")
            fb2_t = pc.tile([128, nw], f32, tag="fb2", name="fb2")
            inp_t = [pin.tile([128, ROWS], bf16, tag=f"inp{k}", name=f"inp{k}") for k in range(2)]
            zt = pc.tile([128, PITCH], bf16, tag="zt", name="zt")
            gw1_t = [pc.tile([128, H], bf16, tag=f"gw1_{k}", name=f"gw1_{k}") for k in range(2)]
            gw2_t = pc.tile([128, HT * EH], bf16, tag="gw2", name="gw2")
            gb1_t = pc.tile([128, HT], f32, tag="gb1", name="gb1")
            gb2_t = pc.tile([1, EH], bf16, tag="gb2", name="gb2")
            ones_t = pc.tile([1, 128], bf16, tag="ones", name="ones")
            vt_t = [pc.tile([128, N], bf16, tag=f"vt{k}", name=f"vt{k}") for k in range(2)]
            vg_sb = pvg.tile([128, NBLK * EH], bf16, tag="vgsb", name="vgsb")

            for k in range(2):
                nc.sync.dma_start(out=inp_t[k][:], in_=inpt[k * 128:(k + 1) * 128, :])
                nc.sync.dma_start(out=gw1_t[k][:], in_=gw1[k * 128:(k + 1) * 128, :])
                nc.scalar.dma_start(out=vt_t[k][:], in_=vtf[k * 128:(k + 1) * 128, :])
            for t in range(HT):
                nc.sync.dma_start(
                    out=gw2_t[:, t * EH:(t + 1) * EH],
                    in_=gw2h[t * 128:(t + 1) * 128, :],
                )
            nc.sync.dma_start(out=fb1_t[:], in_=fb1t[:])
            nc.sync.dma_start(out=fb2_t[:], in_=fb2r[:])
            nc.sync.dma_start(out=gb1_t[:], in_=gb1t[:])
            nc.sync.dma_start(out=gb2_t[:], in_=gb2h[:])
            nc.sync.dma_start(out=ones_t[:], in_=onesr[:])

            # zero the slot-0/1 staging images once (diagonal rewrites never
            # touch the off-diagonal zeros again); scalar HWDGE ring so the
            # 4 MB of zero-writes don't block the f-MLP weight loads
            nc.vector.memset(zt[:], 0.0)
            for par in range(2):
                for k in range(2):
                    nc.scalar.dma_start(
                        out=stage[par][k * 128 * PITCH:(k + 1) * 128 * PITCH].rearrange(
                            "(p f) -> p f", f=PITCH
                        ),
                        in_=zt[:],
                    )

            def g_chunk(ch):
                fh = [pgh.tile([128, CH], bf16, tag=f"gh{t}", name=f"gh{t}") for t in range(HT)]
                for ht in range(HT):
                    pa = psA.tile([128, CH], f32, tag="pa", name="pa")
                    for kt in range(2):
                        nc.tensor.matmul(
                            pa[:],
                            lhsT=gw1_t[kt][:, ht * 128:(ht + 1) * 128],
                            rhs=vt_t[kt][:, ch * CH:(ch + 1) * CH],
                            start=(kt == 0),
                            stop=(kt == 1),
                        )
                    nc.scalar.activation(
                        fh[ht][:], pa[:], AF.Gelu, bias=gb1_t[:, ht:ht + 1]
                    )
                for t in range(4):
                    po = psG.tile([128, EH], f32, tag="pog", name="pog")
                    nc.tensor.matmul(
                        po[:], lhsT=ones_t[0:1, 0:128], rhs=gb2_t[0:1, :],
                        start=True, stop=False,
                    )
                    for ht in range(HT):
                        nc.tensor.matmul(
                            po[:],
                            lhsT=fh[ht][:, t * 128:(t + 1) * 128],
                            rhs=gw2_t[:, ht * EH:(ht + 1) * EH],
                            start=False,
                            stop=(ht == HT - 1),
                        )
                    blk_i = ch * 4 + t
                    nc.vector.tensor_copy(vg_sb[:, blk_i * EH:(blk_i + 1) * EH], po[:])

            for m in range(nw):
                w1 = [pfw1.tile([128, H], bf16, tag=f"fw1_{k}", name=f"fw1_{k}") for k in range(2)]
                for k in range(2):
                    nc.sync.dma_start(out=w1[k][:], in_=fw1[m, k * 128:(k + 1) * 128, :])
                w2 = pfw2.tile([128, HT * NL], bf16, tag="fw2", name="fw2")
                nc.sync.dma_start(out=w2[:], in_=fw2t[m])
                fh = [
                    [pfh.tile([128, CH], bf16, tag=f"fh{ch}_{t}", name=f"fh{ch}_{t}") for t in range(HT)]
                    for ch in range(NCH)
                ]
                for ch in range(NCH):
                    for ht in range(HT):
                        pa = psA.tile([128, CH], f32, tag="pa", name="pa")
                        for kt in range(2):
                            nc.tensor.matmul(
                                pa[:],
                                lhsT=w1[kt][:, ht * 128:(ht + 1) * 128],
                                rhs=inp_t[kt][:, ch * CH:(ch + 1) * CH],
                                start=(kt == 0),
                                stop=(kt == 1),
                            )
                        nc.scalar.activation(
                            fh[ch][ht][:], pa[:], AF.Gelu,
                            bias=fb1_t[:, m * HT + ht:m * HT + ht + 1],
                        )
                # all 4 chunks' second matmuls share the PE via column tiling:
                # chunk j accumulates into PSUM partitions [32j, 32j+13)
                pw = psW.tile([128, CH], f32, tag="pw", name="pw")
                for ht in range(HT):
                    for j in range(NCH):
                        nc.tensor.matmul(
                            pw[32 * j:32 * j + NL, :],
                            lhsT=w2[:, ht * NL:(ht + 1) * NL],
                            rhs=fh[j][ht][:],
                            start=(ht == 0),
                            stop=(ht == HT - 1),
                            tile_position=(0, 32 * j),
                        )
                wc = ptmp.tile([128, CH], bf16, tag="tw", name="tw")
                nc.vector.tensor_scalar_add(wc[:], pw[:], fb2_t[:, m:m + 1])
                for j in range(NCH):
                    nc.sync.dma_start(
                        out=wsis_all[m][:, j * CH:(j + 1) * CH],
                        in_=wc[32 * j:32 * j + NL, :],
                    )
                # interleave one g-MLP chunk per early f layer
                if m < NCHG:
                    g_chunk(m)
            for m in range(nw, NCHG):  # K_NW < 8 debug path
                g_chunk(m)
            nc.scalar.dma_start(out=vg[:], in_=vg_sb[:])

    # ---------------- raw pair-AllGathers (issue only; wait in phase 2) ---
    with nc.semaphore("ag_sem") as ag_sem:
        with nc.Block() as blk:
            @blk.gpsimd
            def _(g):
                g.collective_compute(
                    "AllGather", mybir.AluOpType.bypass, replica_groups=GROUPS,
                    ins=[wsis_all[0:split]], outs=[wsos_a[:]],
                ).then_inc(ag_sem)
                if wsos_b is not None:
                    g.collective_compute(
                        "AllGather", mybir.AluOpType.bypass, replica_groups=GROUPS,
                        ins=[wsis_all[split:nw]], outs=[wsos_b[:]],
                    ).then_inc(ag_sem)

        nc.all_engine_barrier()

        # ---------------- phase 2: chord chain ----------------------------
        agb_waiter = []
        with TileContext(nc) as tc:
            with (
                tc.tile_pool(name="pc2", bufs=1) as pc2,
                tc.tile_pool(name="pvb", bufs=1) as pvb,
                tc.tile_pool(name="pvo", bufs=2) as pvo,
                tc.tile_pool(name="ps", bufs=2) as ps,
                tc.tile_pool(name="pd", bufs=2) as pd,
                tc.tile_pool(name="pw2", bufs=2) as pw2,
                tc.tile_pool(name="psC", bufs=1, space="PSUM") as psC,
            ):
                idm_t = pc2.tile([128, 128], bf16, tag="idm", name="idm")
                nc.sync.dma_start(out=idm_t[:], in_=idmr[:])
                # V from phase 1, per evac group (layer-0 rhs)
                vbf = [
                    [pvb.tile([128, 4 * EH], bf16, tag=f"vb{p}_{g}", name=f"vb{p}_{g}") for g in range(NG)]
                    for p in range(2)
                ]
                for g in range(NG):
                    eng = nc.sync if g % 2 == 0 else nc.scalar
                    eng.dma_start(
                        out=vbf[0][g][:], in_=vg[:, g * 4 * EH:(g + 1) * 4 * EH]
                    )

                def prep_s1(m):
                    """Stage 1 for layer m: W load + interleave + skew writes
                    + diag weight grid spreads. Ring m%2."""
                    st = stage[m % 2]
                    eng = nc.sync if m % 2 == 0 else nc.scalar
                    wt1 = pw2.tile([NL, N], bf16, tag="wt1", name="wt1")
                    Wt = pw2.tile([NL, N], bf16, tag="wt", name="wt")
                    WG = pw2.tile([128, 4 * NBLK], bf16, tag="wg", name="wg")
                    # load this layer's W plain, then (j, b)-interleave:
                    # Wt[l, j*32 + b] = W[128*b + j, l]
                    for h2 in range(2):
                        wsrc = wsos_a[h2, m] if m < split else wsos_b[h2, m - split]
                        inst = eng.dma_start(
                            out=wt1[:, h2 * ROWS:(h2 + 1) * ROWS], in_=wsrc
                        )
                        if h2 == 0 and m in (0, 1, split, split + 1):
                            agb_waiter.append((inst, 1 if m < split else 2))
                    src_il = wt1[:].rearrange("l (b j) -> l j b", j=128)
                    dst_il = Wt[:].rearrange("l (j b) -> l j b", b=NBLK)
                    half = 64
                    nc.vector.tensor_copy(dst_il[:, :half, :], src_il[:, :half, :])
                    nc.gpsimd.tensor_copy(dst_il[:, half:, :], src_il[:, half:, :])
                    # weight grid for the 4 diagonal links (d=256..2048):
                    # WG[p, i*32+b] = w_{9+i}[b*128+p] (SBUF->SBUF row spread)
                    for i in range(4):
                        eng.dma_start(
                            out=WG[:, i * NBLK:(i + 1) * NBLK],
                            in_=Wt[9 + i:10 + i, :].rearrange("o (p b) -> o p b", b=NBLK),
                        )
                    # rewrite the 9 low-link diagonals of the staged S image
                    # (link 0 carries the +1 residual, folded into fb2r on host)
                    for li in range(9):
                        d = OFFS[li]
                        segs = []
                        if 128 - d > 0:
                            segs.append((0, 0, 128 - d, d))
                        if d > 0:
                            segs.append((1, 128 - d, d, 0))
                        for (si, j0, cnt, p0) in segs:
                            src = Wt[li:li + 1, j0 * NBLK:(j0 + cnt) * NBLK]
                            doff = si * 128 * PITCH + p0 * PITCH + j0 * NBLK
                            dst = st[doff:doff + 1]
                            dst.ap = V64([[PITCH + NBLK, cnt], [1, NBLK]])
                            eng.dma_start(out=dst, in_=src)
                    return WG

                def prep_s2(m, WG):
                    """Stage 2 for layer m: S reloads (ring m%2, FIFO after
                    the skew writes) + DVE/GpSimd diagonal builds."""
                    st = stage[m % 2]
                    eng = nc.sync if m % 2 == 0 else nc.scalar
                    Sp = []
                    for k in range(2):
                        s = ps.tile([128, PITCH], bf16, tag=f"s{k}", name=f"s{k}")
                        eng.dma_start(
                            out=s[:],
                            in_=st[k * 128 * PITCH:(k + 1) * 128 * PITCH].rearrange(
                                "(p f) -> p f", f=PITCH
                            ),
                        )
                        Sp.append(s)
                    # build the 4 diagonal lhsT tiles:
                    # D[p, (b, j)] = id[p, j] * WG[p, i*32+b]
                    Dp = []
                    for i in range(4):
                        dt_ = pd.tile([128, PITCH], bf16, tag=f"d{i}", name=f"d{i}")
                        veng = nc.vector if i < 2 else nc.gpsimd
                        veng.tensor_mul(
                            dt_[:].rearrange("p (b j) -> p b j", j=128),
                            idm_t[:, None, :].broadcast_to((128, NBLK, 128)),
                            WG[:, i * NBLK:(i + 1) * NBLK][:, :, None].broadcast_to(
                                (128, NBLK, 128)
                            ),
                        )
                        Dp.append(dt_)
                    return Sp + Dp

                def compute(m, lhs_all, last):
                    vin = vbf[m % 2]
                    vout = vbf[(m + 1) % 2]
                    boffs = [0, 1] + DIAG_BOFF
                    for g4 in range(NG):
                        po = psC.tile(
                            [128, 4 * EH], f32, tag=f"poc{g4}", name=f"poc{g4}"
                        )
                        for i4 in range(4):
                            blk_i = g4 * 4 + i4
                            for ii in range(6):
                                sb = (blk_i + boffs[ii]) % NBLK
                                if ii < 2:
                                    lhsT = lhs_all[ii][:, blk_i::NBLK]
                                else:
                                    lhsT = lhs_all[ii][:, blk_i * 128:(blk_i + 1) * 128]
                                nc.tensor.matmul(
                                    po[:, i4 * EH:(i4 + 1) * EH],
                                    lhsT=lhsT,
                                    rhs=vin[sb // 4][:, (sb % 4) * EH:(sb % 4 + 1) * EH],
                                    start=(ii == 0),
                                    stop=(ii == 5),
                                )
                        if not last:
                            nc.scalar.copy(vout[g4][:], po[:])
                        else:
                            vo = pvo.tile([128, 4 * EH], f32, tag="vo", name="vo")
                            nc.scalar.copy(vo[:], po[:])
                            eng = nc.sync if g4 % 2 == 0 else nc.scalar
                            eng.dma_start(
                                out=out[:].rearrange("(b p) e -> p b e", p=128)[
                                    :, g4 * 4:(g4 + 1) * 4, :
                                ],
                                in_=vo[:].rearrange("p (b e) -> p b e", e=EH),
                            )

                # prologue: 2-stage pipeline fill
                wg0 = prep_s1(0)
                wg1 = prep_s1(1) if nw > 1 else None
                lhs0 = prep_s2(0, wg0)
                lhs = lhs0
                wgs = {1: wg1}
                for m in range(nw):
                    if m + 2 < nw:
                        wgs[m + 2] = prep_s1(m + 2)
                    nxt = prep_s2(m + 1, wgs.pop(m + 1)) if m + 1 < nw else None
                    compute(m, lhs, last=(m == nw - 1))
                    lhs = nxt

        # gate each ring's first wsos readers on their AllGather (an
        # in-context wait on an externally-signaled sem would deadlock the
        # Tile scheduling simulator)
        for inst, val in agb_waiter:
            inst.wait_op(ag_sem, val, "sem-ge", check=False)

    _split_multi_waits(nc, mybir)
    return nc


def kernel(**inputs):
    _install_patches()
    from concourse.bass_utils import run_bass_kernel_spmd

    nw = int(os.environ.get("K_NW", NW))
    V = np.ascontiguousarray(np.asarray(inputs["V"], dtype=np.float32))
    inp = np.ascontiguousarray(np.asarray(inputs["input"], dtype=np.float32))
    g_W1 = np.ascontiguousarray(np.asarray(inputs["g_W1"], dtype=np.float32))
    g_b1 = np.asarray(inputs["g_b1"], dtype=np.float32)
    g_W2 = np.ascontiguousarray(np.asarray(inputs["g_W2"], dtype=np.float32))
    g_b2 = np.asarray(inputs["g_b2"], dtype=np.float32)
    f_W1 = np.ascontiguousarray(np.asarray(inputs["f_W1"], dtype=np.float32))[:nw]
    f_b1 = np.asarray(inputs["f_b1"], dtype=np.float32)[:nw]
    f_W2 = np.ascontiguousarray(np.asarray(inputs["f_W2"], dtype=np.float32))[:nw]
    f_b2 = np.asarray(inputs["f_b2"], dtype=np.float32)[:nw]

    import ml_dtypes

    bf = ml_dtypes.bfloat16
    gb1t = np.ascontiguousarray(g_b1.reshape(HT, 128).T)
    fw2t = np.ascontiguousarray(
        f_W2.reshape(nw, HT, 128, NL).transpose(0, 2, 1, 3).reshape(nw, 128, HT * NL)
    ).astype(bf)
    fb1t = np.ascontiguousarray(
        f_b1.reshape(nw, HT, 128).transpose(2, 0, 1).reshape(128, nw * HT)
    )
    fb2c = np.ascontiguousarray(f_b2.T).copy()
    fb2c[0, :] += 1.0  # fold the +V residual into the self-link weight
    # replicate over the 4 column-tile groups: fb2r[32j+l, m] = fb2c[l, m]
    fb2r = np.zeros((128, nw), np.float32)
    for j in range(4):
        fb2r[32 * j:32 * j + NL, :] = fb2c

    shared = {
        "gw1": g_W1.astype(bf),
        "gb1t": gb1t,
        "onesr": np.ones((1, 128), bf),
        "idmr": np.eye(128, dtype=bf),
        "fw1": f_W1.astype(bf),
        "fw2t": fw2t,
        "fb1t": fb1t,
        "fb2r": fb2r,
    }
    in_maps = []
    for c in range(8):
        b, h = c // 2, c % 2
        rows = slice(h * ROWS, (h + 1) * ROWS)
        ecols = slice(h * EH, (h + 1) * EH)
        m = dict(shared)
        m["vtf"] = np.ascontiguousarray(V[b].T).astype(bf)
        m["inpt"] = np.ascontiguousarray(inp[b, rows].T).astype(bf)
        m["gw2h"] = np.ascontiguousarray(g_W2[:, ecols]).astype(bf)
        m["gb2h"] = np.ascontiguousarray(g_b2[None, ecols]).astype(bf)
        in_maps.append(m)

    nc = _build_program(nw)
    trace = bool(int(os.environ.get("K_TRACE", "0")))
    res = run_bass_kernel_spmd(nc, in_maps, list(range(8)), trace=trace)
    kernel.last_result = res

    outp = np.empty((B, N, E), np.float32)
    for b in range(B):
        outp[b, :, :EH] = res.results[2 * b]["out"]
        outp[b, :, EH:] = res.results[2 * b + 1]["out"]
    return outp
